# revision 14
# baseline (speedup 1.0000x reference)
"""Trainium2 Bass kernel: multi-head attention (transposed-causal softmax).

Reference math (B=4, N=2048, D=1024, H=16, E=64):
    qkv = x @ W_qkv -> split (3, H, E)
    scores[i, j] = k_i . q_j / sqrt(E)          (i = key pos, j = query pos)
    mask: keep i <= j; softmax over j; out[i] = sum_j attn[i, j] v_j
    y = concat_heads(out) @ W_o

Sharding (8 cores): data-parallel over batch (4) x tensor-parallel over
head-groups (2 groups of 8 heads). Each core computes a full [N, D] partial
projection output for its (batch, head-group); the host sums group pairs.

Per-core layout strategy (all matmul inputs bf16, fp32 PSUM accumulation):
  - host supplies xT [D, N] so QKV projections contract over D on partitions
  - scores are built transposed: S^T[j, i] = q'_j . k_i (scale folded in Wq),
    j on partitions -> softmax sum over j comes free from the AV matmul by
    augmenting V with a ones column (Z lands in PSUM row 64)
  - heads processed in pairs: head A lives on partitions 0-63, head B on
    64-127, so K=64 score matmuls for both heads run concurrently in
    disjoint PE row groups
  - causal structure: j-tiles iterate descending so PSUM accumulation starts
    with the full-width tile; fully-masked tiles are skipped; diagonal tiles
    multiply a lower-triangular mask into exp(S^T)
  - 1/Z via fast DVE reciprocal on a batched [heads*chunks, 512] tile;
    broadcast across partitions on GPSIMD; O^T normalized in SBUF
  - final projection contracts head pairs straight out of the O^T layout
"""

import os
import sys
from contextlib import ExitStack

import numpy as np

for _p in ("/opt/trn_rl_repo",):
    if os.path.isdir(_p) and _p not in sys.path:
        sys.path.insert(0, _p)

import ml_dtypes

import concourse.bacc as bacc
import concourse.mybir as mybir
import concourse.tile as tile
from concourse.bass_utils import run_bass_kernel_spmd
from concourse.masks import make_identity, make_lower_triangular

AF = mybir.ActivationFunctionType
F32 = mybir.dt.float32
BF16 = mybir.dt.bfloat16
BF16NP = ml_dtypes.bfloat16

B, N, D, H, E = 4, 2048, 1024, 16, 64
N_CORES = 8
HPC = H // 2  # heads per core (tensor-parallel over 2 head groups)
CHW = 512     # i-chunk width (one fp32 PSUM bank)
VBLK = 192    # V-natural block stride: [V_A(64) | 1 | pad | V_B(64) | 1 | pad]


def emit_attention(ctx, tc, y, xt, wq, wk, wv, wo, n, d, hpc, dbg=None):
    """Emit the per-core kernel body into TileContext `tc`.

    y:  [n, d] f32 out;  xt: [d, n] bf16;  wq/wk/wv: [d, hpc*64] bf16
    (wq pre-scaled by 1/sqrt(E));  wo: [hpc*64, d] bf16.
    """
    nc = tc.nc
    KT = d // 128        # contraction tiles for projections
    NT = n // 128        # j-tiles
    NCH = n // CHW       # i-chunks
    NP = hpc // 2        # head pairs
    DQ = hpc * 64        # per-core q/k/v width
    OC = min(512, d)     # out-projection column chunk
    NOC = d // OC
    TPC = CHW // 128     # j-tiles per chunk width (4)

    # constants
    cpool = ctx.enter_context(tc.tile_pool(name="consts", bufs=1))
    ident = cpool.tile([128, 128], BF16, tag="ident", name="ident")
    make_identity(nc, ident)
    tri = cpool.tile([128, 128], BF16, tag="tri", name="tri")
    make_lower_triangular(nc, tri, val=1.0, diag=True)
    ones1 = cpool.tile([1, 128], BF16, tag="ones1", name="ones1")
    nc.gpsimd.memset(ones1, 1.0)
    if dbg is not None:
        nc.sync.dma_start(dbg["tri"], tri)
        nc.sync.dma_start(dbg["ident"], ident)

    # persistent SBUF tensors
    big = ctx.enter_context(tc.tile_pool(name="big", bufs=1))
    xt_sb = big.tile([128, KT * n], BF16, tag="xt", name="xt_sb")
    nc.sync.dma_start(
        xt_sb.rearrange("p (k c) -> p k c", k=KT),
        xt.rearrange("(k p) c -> p k c", p=128),
    )
    w_sbs = []
    for nm, wd in (("wq", wq), ("wk", wk), ("wv", wv)):
        w_sb = big.tile([128, KT * DQ], BF16, tag=nm, name=nm + "_sb")
        nc.sync.dma_start(
            w_sb.rearrange("p (k c) -> p k c", k=KT),
            wd.rearrange("(k p) c -> p k c", p=128),
        )
        w_sbs.append(w_sb)
    DT = DQ // 128  # de-tiles for out projection (== NP)
    wo_sb = big.tile([128, DT * d], BF16, tag="wo", name="wo_sb")
    nc.sync.dma_start(
        wo_sb.rearrange("p (t c) -> p t c", t=DT),
        wo.rearrange("(t p) c -> p t c", p=128),
    )
    ot_all = []
    for p_ in range(NP):
        t_ = big.tile([128, n], BF16, tag=f"ot{p_}", name=f"ot{p_}")
        ot_all.append(t_)
    z_all = big.tile([hpc * NCH, CHW], BF16, tag="z", name="z_all")

    # working pools
    qkvp = ctx.enter_context(tc.tile_pool(name="qkv", bufs=2))
    ptp = ctx.enter_context(tc.tile_pool(name="pt", bufs=2))
    stp = ctx.enter_context(tc.tile_pool(name="st", bufs=2))
    ystp = ctx.enter_context(tc.tile_pool(name="yst", bufs=4))
    zbp = ctx.enter_context(tc.tile_pool(name="zb", bufs=2))
    npool = ctx.enter_context(tc.tile_pool(name="nrm", bufs=1))
    psb = ctx.enter_context(tc.tile_pool(name="psb", bufs=2, space="PSUM"))
    pss = ctx.enter_context(tc.tile_pool(name="pss", bufs=2, space="PSUM"))
    pso = ctx.enter_context(tc.tile_pool(name="pso", bufs=1, space="PSUM"))

    for p_ in range(NP):
        # --- QKV projection for this head pair (transposed outputs) ---
        qt = qkvp.tile([128, n], BF16, tag="qt", name=f"qt{p_}")
        kt = qkvp.tile([128, n], BF16, tag="kt", name=f"kt{p_}")
        vt = qkvp.tile([128, n], BF16, tag="vt", name=f"vt{p_}")
        vna = qkvp.tile([128, NT * VBLK], BF16, tag="vna", name=f"vna{p_}")
        for w_sb, dst in zip(w_sbs, (qt, kt, vt)):
            for chn in range(n // 512):
                ps = psb.tile([128, 512], F32, tag="big", name="ps_qkv")
                for k_ in range(KT):
                    nc.tensor.matmul(
                        ps,
                        lhsT=w_sb[:, k_ * DQ + p_ * 128 : k_ * DQ + (p_ + 1) * 128],
                        rhs=xt_sb[:, k_ * n + chn * 512 : k_ * n + chn * 512 + 512],
                        start=(k_ == 0),
                        stop=(k_ == KT - 1),
                    )
                nc.vector.tensor_copy(dst[:, chn * 512 : (chn + 1) * 512], ps)
        # --- V natural layout (+ ones column for Z) via PE transpose ---
        nc.gpsimd.memset(vna, 1.0)
        for t_ in range(NT):
            pst = psb.tile([128, 128], BF16, tag="big", name="ps_tr")
            nc.tensor.transpose(pst, vt[:, t_ * 128 : (t_ + 1) * 128], ident)
            nc.vector.tensor_copy(vna[:, t_ * VBLK : t_ * VBLK + 64], pst[:, 0:64])
            nc.vector.tensor_copy(
                vna[:, t_ * VBLK + 96 : t_ * VBLK + 160], pst[:, 64:128]
            )
        if dbg is not None and p_ == 0:
            nc.sync.dma_start(dbg["qt"], qt)
            nc.sync.dma_start(dbg["kt"], kt)
            nc.sync.dma_start(dbg["vna"], vna)

        # --- attention (both heads of the pair) ---
        for cc in range(NCH):
            poa = pso.tile([65, CHW], F32, tag="oA", name="poa")
            pob = pso.tile([65, CHW], F32, tag="oB", name="pob")
            for t_ in range(NT - 1, TPC * cc - 1, -1):
                o = 128 * t_ - CHW * cc
                w = min(CHW, o + 128)
                first = t_ == NT - 1
                last = t_ == TPC * cc
                psa = pss.tile([128, CHW], F32, tag="sA", name="psa")
                psb_ = pss.tile([128, CHW], F32, tag="sB", name="psb")
                nc.tensor.matmul(
                    psa[:, :w],
                    lhsT=qt[0:64, t_ * 128 : (t_ + 1) * 128],
                    rhs=kt[0:64, cc * CHW : cc * CHW + w],
                    start=True,
                    stop=True,
                )
                nc.tensor.matmul(
                    psb_[:, :w],
                    lhsT=qt[64:128, t_ * 128 : (t_ + 1) * 128],
                    rhs=kt[64:128, cc * CHW : cc * CHW + w],
                    start=True,
                    stop=True,
                )
                pa = ptp.tile([128, CHW], BF16, tag="pA", name="pa")
                pb = ptp.tile([128, CHW], BF16, tag="pB", name="pb")
                nc.scalar.activation(pa[:, :w], psa[:, :w], AF.Exp)
                nc.scalar.activation(pb[:, :w], psb_[:, :w], AF.Exp)
                if o < CHW:  # diagonal tile: keep i <= j within the block
                    nc.vector.tensor_mul(pa[:, o : o + 128], pa[:, o : o + 128], tri)
                    nc.vector.tensor_mul(pb[:, o : o + 128], pb[:, o : o + 128], tri)
                nc.tensor.matmul(
                    poa[:, :w],
                    lhsT=vna[:, t_ * VBLK : t_ * VBLK + 65],
                    rhs=pa[:, :w],
                    start=first,
                    stop=last,
                    skip_group_check=True,
                )
                nc.tensor.matmul(
                    pob[:, :w],
                    lhsT=vna[:, t_ * VBLK + 96 : t_ * VBLK + 161],
                    rhs=pb[:, :w],
                    start=first,
                    stop=last,
                    skip_group_check=True,
                )
            # evacuate O^T (+Z row) and shift head B to partitions 64-127
            sta = stp.tile([65, CHW], BF16, tag="stA", name="sta")
            stb = stp.tile([65, CHW], BF16, tag="stB", name="stb")
            nc.vector.tensor_copy(sta, poa)
            nc.vector.tensor_copy(stb, pob)
            nc.sync.dma_start(ot_all[p_][0:64, cc * CHW : (cc + 1) * CHW], sta[0:64, :])
            nc.sync.dma_start(
                ot_all[p_][64:128, cc * CHW : (cc + 1) * CHW], stb[0:64, :]
            )
            ra = (2 * p_) * NCH + cc
            rb = (2 * p_ + 1) * NCH + cc
            nc.sync.dma_start(z_all[ra : ra + 1, :], sta[64:65, :])
            nc.sync.dma_start(z_all[rb : rb + 1, :], stb[64:65, :])
            if dbg is not None and p_ == 0:
                nc.sync.dma_start(
                    dbg["ot0pre"][0:64, cc * CHW : (cc + 1) * CHW], sta[0:64, :]
                )
                nc.sync.dma_start(
                    dbg["ot0pre"][64:128, cc * CHW : (cc + 1) * CHW], stb[0:64, :]
                )

    # --- softmax normalization: O^T *= 1/Z ---
    zf = npool.tile([hpc * NCH, CHW], F32, tag="zf", name="zf")
    nc.vector.tensor_copy(zf, z_all)
    zinv = npool.tile([hpc * NCH, CHW], F32, tag="zinv", name="zinv")
    nc.vector.reciprocal_approx_fast(zinv, zf)
    zinv_bf = npool.tile([hpc * NCH, CHW], BF16, tag="zinv_bf", name="zinv_bf")
    nc.vector.tensor_copy(zinv_bf, zinv)
    if dbg is not None:
        nc.sync.dma_start(dbg["z"], z_all)
        nc.sync.dma_start(dbg["zinv"], zinv)
    for p_ in range(NP):
        for cc in range(NCH):
            # broadcast 1/Z across partitions with a K=1 matmul
            zra = zbp.tile([1, CHW], BF16, tag="zra", name="zra")
            zrb = zbp.tile([1, CHW], BF16, tag="zrb", name="zrb")
            ra = (2 * p_) * NCH + cc
            rb = (2 * p_ + 1) * NCH + cc
            nc.sync.dma_start(zra, zinv_bf[ra : ra + 1, :])
            nc.sync.dma_start(zrb, zinv_bf[rb : rb + 1, :])
            zb = psb.tile([128, CHW], F32, tag="big", name="zb")
            nc.tensor.matmul(zb[0:64, :], lhsT=ones1[:, 0:64], rhs=zra, start=True, stop=True)
            nc.tensor.matmul(zb[64:128, :], lhsT=ones1[:, 0:64], rhs=zrb, start=True, stop=True)
            nc.vector.tensor_mul(
                ot_all[p_][:, cc * CHW : (cc + 1) * CHW],
                ot_all[p_][:, cc * CHW : (cc + 1) * CHW],
                zb,
            )
    if dbg is not None:
        nc.sync.dma_start(dbg["ot0"], ot_all[0])

    # --- output projection: y[i, :] = sum_p OT_p[:, i].T @ wo_p ---
    for it in range(NT):
        for hf in range(NOC):
            pf = psb.tile([128, OC], F32, tag="big", name="pf")
            for p_ in range(NP):
                nc.tensor.matmul(
                    pf,
                    lhsT=ot_all[p_][:, it * 128 : (it + 1) * 128],
                    rhs=wo_sb[:, p_ * d + hf * OC : p_ * d + hf * OC + OC],
                    start=(p_ == 0),
                    stop=(p_ == NP - 1),
                )
            ys = ystp.tile([128, OC], F32, tag="y", name="ys")
            nc.vector.tensor_copy(ys, pf)
            nc.sync.dma_start(y[it * 128 : (it + 1) * 128, hf * OC : (hf + 1) * OC], ys)


def build_nc(n=N, d=D, hpc=HPC, num_devices=N_CORES, enable_asserts=False, debug_outs=False):
    nc = bacc.Bacc(
        "TRN2",
        target_bir_lowering=False,
        debug=False,
        enable_asserts=enable_asserts,
        num_devices=num_devices,
    )
    dq = hpc * 64
    xt = nc.dram_tensor("xt", [d, n], BF16, kind="ExternalInput").ap()
    wq = nc.dram_tensor("wq", [d, dq], BF16, kind="ExternalInput").ap()
    wk = nc.dram_tensor("wk", [d, dq], BF16, kind="ExternalInput").ap()
    wv = nc.dram_tensor("wv", [d, dq], BF16, kind="ExternalInput").ap()
    wo = nc.dram_tensor("wo", [dq, d], BF16, kind="ExternalInput").ap()
    y = nc.dram_tensor("y", [n, d], F32, kind="ExternalOutput").ap()
    dbg = None
    if debug_outs:
        NT_, NCH_ = n // 128, n // CHW
        dbg = {
            "tri": nc.dram_tensor("dbg_tri", [128, 128], BF16, kind="ExternalOutput").ap(),
            "ident": nc.dram_tensor("dbg_ident", [128, 128], BF16, kind="ExternalOutput").ap(),
            "qt": nc.dram_tensor("dbg_qt", [128, n], BF16, kind="ExternalOutput").ap(),
            "kt": nc.dram_tensor("dbg_kt", [128, n], BF16, kind="ExternalOutput").ap(),
            "vna": nc.dram_tensor("dbg_vna", [128, NT_ * VBLK], BF16, kind="ExternalOutput").ap(),
            "z": nc.dram_tensor("dbg_z", [hpc * NCH_, CHW], BF16, kind="ExternalOutput").ap(),
            "zinv": nc.dram_tensor("dbg_zinv", [hpc * NCH_, CHW], F32, kind="ExternalOutput").ap(),
            "ot0pre": nc.dram_tensor("dbg_ot0pre", [128, n], BF16, kind="ExternalOutput").ap(),
            "ot0": nc.dram_tensor("dbg_ot0", [128, n], BF16, kind="ExternalOutput").ap(),
        }
    with tile.TileContext(nc) as tc, ExitStack() as ctx:
        emit_attention(ctx, tc, y, xt, wq, wk, wv, wo, n, d, hpc, dbg=dbg)
    nc.compile()
    return nc


def make_in_maps(x, W_qkv, W_o):
    scale = np.float32(1.0 / np.sqrt(E))
    dq = HPC * 64
    in_maps = []
    for c in range(N_CORES):
        b, g = divmod(c, 2)
        in_maps.append(
            {
                "xt": np.ascontiguousarray(x[b].T).astype(BF16NP),
                "wq": (W_qkv[:, g * dq : (g + 1) * dq] * scale).astype(BF16NP),
                "wk": np.ascontiguousarray(
                    W_qkv[:, D + g * dq : D + (g + 1) * dq]
                ).astype(BF16NP),
                "wv": np.ascontiguousarray(
                    W_qkv[:, 2 * D + g * dq : 2 * D + (g + 1) * dq]
                ).astype(BF16NP),
                "wo": np.ascontiguousarray(W_o[g * dq : (g + 1) * dq, :]).astype(
                    BF16NP
                ),
            }
        )
    return in_maps


_NC_CACHE = {}


def kernel(x, W_qkv, W_o):
    x = np.asarray(x, dtype=np.float32)
    W_qkv = np.asarray(W_qkv, dtype=np.float32)
    W_o = np.asarray(W_o, dtype=np.float32)
    if "nc" not in _NC_CACHE:
        _NC_CACHE["nc"] = build_nc()
    in_maps = make_in_maps(x, W_qkv, W_o)
    res = run_bass_kernel_spmd(_NC_CACHE["nc"], in_maps, list(range(N_CORES)))
    ys = [np.asarray(res.results[i]["y"], dtype=np.float32) for i in range(N_CORES)]
    return np.stack([ys[2 * b] + ys[2 * b + 1] for b in range(B)])
